# revision 44
# baseline (speedup 1.0000x reference)
"""Trainium2 Bass kernel: batched int8 GEMM (bmm_s8t_s8n) with fused bf16 dequant.

Computes out[i] = bf16(alpha * (a[i] @ b[i]^T)) for a [32,512,2048] int8,
b [32,512,2048] int8 (both row-major with K innermost), alpha scalar fp32.

Strategy (per 8-core SPMD shard = 4 batches/core):
  1. The TensorE contracts over the partition axis, so both operands need K on
     partitions.  DMA xbar transpose only supports 2-byte elements, so we view
     int8 pairs along K as uint16 and transpose pieces of each [512, 1024]u16
     batch matrix into SBUF tiles [128, chunks, 512]u16.  A partition then
     holds two int8 k-slices byte-interleaved along the free dim.  Whatever
     (partition, chunk) <-> column mapping the xbar uses, it is identical for
     a and b, so the contraction enumerates every k exactly once — correctness
     does not depend on the mapping.
  2. int8 -> bf16 conversion copies (DVE / ACT) de-interleave the two parities
     into a [128, 1024] bf16 tile whose halves are two k-slices.  int8 values
     are exact in bf16; products accumulate exactly in fp32 PSUM
     (|acc| << 2^24), so results match int32 accumulation bit-wise.
  3. 16 accumulating matmuls per output tile: psum[128m, 512n] += aT.T @ bT,
     issued k-tile-major across the 4 m-groups (4 open PSUM banks) so the PE
     consumes each arriving k-tile with 4 back-to-back matmuls and never
     starves during the conversion trickle-in.
  4. Dequant: DVE tensor_scalar PSUM->SBUF with scale=alpha, cast bf16, DMA out.

Measured (8 cores, NTFF profile): ~80 us end-to-end; PE dense region ~57 us
(~233 ns per N=512 matmul vs the 213 ns stream floor), bitwise-exact output.
"""

from dataclasses import dataclass, replace

import numpy as np

import concourse.mybir as mybir
from concourse import bacc
from concourse.bass_utils import run_bass_kernel_spmd
from concourse.tile import TileContext

B, M, N, K = 32, 512, 512, 2048
NCORES = 8
BPC = B // NCORES  # batches per core
KP = K // 2  # uint16 pair-columns per row
PART = 128
NCHUNK = KP // PART  # transposed chunks per operand-batch (8)
KTILES = 2 * NCHUNK  # k-tiles of 128 per batch (16)


@dataclass(frozen=True)
class Cfg:
    stage_bufs: int = 3  # per tag [128, 4096]u16 staging tiles (8 KiB/partition)
    conv_bufs: int = 20  # per tag [128, 1024]bf16 k-tiles (2 KiB/partition)
    split0: int = 2  # transpose pieces for batch 0
    split: int = 2  # transpose pieces for later batches
    alt0: bool = False  # alternate conv engines on batch 0
    store_eng: str = "gpsimd"
    conv_chunks: int = 1  # chunks converted per copy instruction
    gpsimd_convs: bool = False  # route some b conversions to GPSIMD
    obuf_bufs: int = 8
    psum_bufs: int = 8
    mm_order: str = "m_major"  # m_major: per m-group over k-tiles; t_major: per k-tile over m-groups
    b_first: bool = False  # issue b transposes before a within each piece


VARIANTS = {
    "v5": Cfg(),
    "v7": Cfg(stage_bufs=4, split0=4, alt0=True),
    "v8": Cfg(split0=4),
    "v9": Cfg(stage_bufs=4),
    "v10": Cfg(split0=1, split=1),
    "v11": Cfg(alt0=True),
    "v12": Cfg(conv_chunks=2, conv_bufs=10),
    "v13": Cfg(conv_bufs=12),
    "v14": Cfg(conv_bufs=16),
    "v15": Cfg(gpsimd_convs=True),
    "v16": Cfg(obuf_bufs=4, psum_bufs=4),
    "v17": Cfg(store_eng="scalar"),
    "v19": Cfg(mm_order="t_major"),
    "v20": Cfg(mm_order="t_major", alt0=True),
    "v21": Cfg(mm_order="t_major", split0=4),
    "v22": Cfg(mm_order="t_major", conv_bufs=24),
    "v23": Cfg(mm_order="t_major_tail"),
    "v24": Cfg(mm_order="t_major", b_first=True),
    "v25": Cfg(mm_order="t_major", split=1),
    "v26": Cfg(mm_order="t_major", stage_bufs=2),
}

_cfg = VARIANTS["v19"]


def set_variant(name):
    global _cfg
    _cfg = VARIANTS[name] if isinstance(name, str) else name


def _build(alpha: float, bpc: int = BPC):
    cfg = _cfg
    nc = bacc.Bacc("TRN2", target_bir_lowering=False)
    a_d = nc.dram_tensor("a", [bpc, M, KP], mybir.dt.uint16, kind="ExternalInput")
    b_d = nc.dram_tensor("b", [bpc, N, KP], mybir.dt.uint16, kind="ExternalInput")
    o_d = nc.dram_tensor("out", [bpc, M, N], mybir.dt.bfloat16, kind="ExternalOutput")

    with TileContext(nc) as tc:
        with (
            tc.tile_pool(name="stage", bufs=cfg.stage_bufs) as stage,
            tc.tile_pool(name="conv", bufs=cfg.conv_bufs) as conv,
            tc.tile_pool(name="obuf", bufs=cfg.obuf_bufs) as obuf,
            tc.tile_pool(name="psum", bufs=cfg.psum_bufs, space="PSUM") as psum_pool,
        ):
            store_ring = getattr(nc, cfg.store_eng)
            for bi in range(bpc):
                ktiles = {"a": [], "b": []}
                sts = {}
                stt = {}
                for name in ("a", "b"):
                    st = stage.tile([PART, NCHUNK * M], mybir.dt.uint16, tag=f"st_{name}")
                    stt[name] = st
                    sts[name] = st[:, :].bitcast(mybir.dt.int8)  # [128, 2*KP]
                # Split transposes into pieces, alternating a/b, so the PE's
                # first k-tiles (which need both operands) arrive sooner.
                pieces = cfg.split0 if bi == 0 else cfg.split
                cs = NCHUNK // pieces  # chunks per piece
                t_order = (("b", b_d), ("a", a_d)) if cfg.b_first else (("a", a_d), ("b", b_d))
                for h in range(pieces):
                    for name, dram in t_order:
                        nc.sync.dma_start_transpose(
                            stt[name][:, h * cs * M : (h + 1) * cs * M].rearrange(
                                "q (c m) -> q c m", m=M
                            ),
                            dram[bi, :, h * cs * PART : (h + 1) * cs * PART],
                        )
                cc = cfg.conv_chunks
                for c0 in range(0, NCHUNK, cc):
                    for name in ("a", "b"):
                        chunk8 = sts[name][:, c0 * 2 * M : (c0 + cc) * 2 * M]
                        if cfg.alt0 and bi == 0:
                            eng = nc.vector if (c0 % 2 == 0) == (name == "a") else nc.scalar
                        elif name == "a":
                            eng = nc.vector
                        elif cfg.gpsimd_convs and c0 % 4 == 3:
                            eng = nc.gpsimd
                        else:
                            eng = nc.scalar
                        bt = conv.tile(
                            [PART, cc * 2 * M], mybir.dt.bfloat16, tag=f"bf_{name}"
                        )
                        # in: [q][c][m][p] bytes -> iterate (c, p, m); out [c][p][m]
                        in_ap = chunk8.rearrange("q (c m p) -> q c p m", p=2, m=M)
                        out_ap = bt[:, :].rearrange("q (c p m) -> q c p m", m=M, p=2)
                        if eng is nc.scalar:
                            eng.copy(out=out_ap, in_=in_ap)
                        else:
                            eng.tensor_copy(out=out_ap, in_=in_ap)
                        for j in range(cc):
                            ktiles[name].append(bt[:, j * 2 * M : (j + 1) * 2 * M])
                n_mt = M // PART

                def mm(ps, mi, c, p, t):
                    nc.tensor.matmul(
                        ps[:, :],
                        ktiles["a"][c][:, p * M + mi * PART : p * M + (mi + 1) * PART],
                        ktiles["b"][c][:, p * N : (p + 1) * N],
                        start=(t == 0),
                        stop=(t == KTILES - 1),
                    )

                def epilogue(ps, mi):
                    ot = obuf.tile([PART, N], mybir.dt.bfloat16)
                    nc.vector.tensor_scalar_mul(ot[:, :], ps[:, :], float(alpha))
                    store_ring.dma_start(o_d[bi, mi * PART : (mi + 1) * PART, :], ot[:, :])

                if cfg.mm_order == "m_major":
                    for mi in range(n_mt):
                        ps = psum_pool.tile([PART, N], mybir.dt.float32)
                        for t in range(KTILES):
                            mm(ps, mi, t // 2, t % 2, t)
                        epilogue(ps, mi)
                else:
                    pss = [
                        psum_pool.tile([PART, N], mybir.dt.float32, name=f"ps_{bi}_{mi}", tag="ps")
                        for mi in range(n_mt)
                    ]
                    for t in range(KTILES - 1):
                        for mi in range(n_mt):
                            mm(pss[mi], mi, t // 2, t % 2, t)
                    t = KTILES - 1
                    if cfg.mm_order == "t_major_tail":
                        # finish each m-group's accumulation and immediately
                        # emit its dequant+store so the epilogues overlap the
                        # remaining groups' final matmuls
                        for mi in range(n_mt):
                            mm(pss[mi], mi, t // 2, t % 2, t)
                            epilogue(pss[mi], mi)
                    else:
                        for mi in range(n_mt):
                            mm(pss[mi], mi, t // 2, t % 2, t)
                        for mi in range(n_mt):
                            epilogue(pss[mi], mi)
    nc.compile()
    return nc


def run(a, b, alpha, trace=False, repeats=1):
    """Run on 8 NeuronCores; returns (out [32,512,512] bf16, list[BassKernelResults])."""
    a = np.ascontiguousarray(np.asarray(a))
    b = np.ascontiguousarray(np.asarray(b))
    if a.dtype != np.int8:
        a = a.astype(np.int8)
    if b.dtype != np.int8:
        b = b.astype(np.int8)
    nc = _build(float(alpha))
    in_maps = []
    for ci in range(NCORES):
        sl = slice(ci * BPC, (ci + 1) * BPC)
        in_maps.append({"a": a[sl].view(np.uint16), "b": b[sl].view(np.uint16)})
    all_res = []
    for _ in range(repeats):
        res = run_bass_kernel_spmd(
            nc, in_maps, core_ids=list(range(NCORES)), trace=trace
        )
        all_res.append(res)
    out = np.concatenate([r["out"] for r in all_res[-1].results], axis=0)
    return out, all_res


def kernel(a, b, alpha):
    out, _ = run(a, b, alpha)
    return out


# revision 51
# speedup vs baseline: 1.0216x; 1.0216x over previous
"""Trainium2 Bass kernel: batched int8 GEMM (bmm_s8t_s8n) with fused bf16 dequant.

Computes out[i] = bf16(alpha * (a[i] @ b[i]^T)) for a [32,512,2048] int8,
b [32,512,2048] int8 (both row-major with K innermost), alpha scalar fp32.

Strategy (per 8-core SPMD shard = 4 batches/core):
  1. The TensorE contracts over the partition axis, so both operands need K on
     partitions.  DMA xbar transpose only supports 2-byte elements, so we view
     int8 pairs along K as uint16 and transpose pieces of each [512, 1024]u16
     batch matrix into SBUF tiles [128, chunks, 512]u16.  A partition then
     holds two int8 k-slices byte-interleaved along the free dim.  Whatever
     (partition, chunk) <-> column mapping the xbar uses, it is identical for
     a and b, so the contraction enumerates every k exactly once — correctness
     does not depend on the mapping.
  2. int8 -> bf16 conversion copies (DVE / ACT) de-interleave the two parities
     into a [128, 1024] bf16 tile whose halves are two k-slices.  int8 values
     are exact in bf16; products accumulate exactly in fp32 PSUM
     (|acc| << 2^24), so results match int32 accumulation bit-wise.
  3. 16 accumulating matmuls per output tile: psum[128m, 512n] += aT.T @ bT,
     issued k-tile-major across the 4 m-groups (4 open PSUM banks) so the PE
     consumes each arriving k-tile with 4 back-to-back matmuls and never
     starves during the conversion trickle-in.  The last batch runs m-major
     instead, so 3 of its 4 dequant+store epilogues hide inside the dense
     matmul region rather than trailing the final matmul.
  4. Dequant: DVE tensor_scalar PSUM->SBUF with scale=alpha, cast bf16, DMA out.

Measured (8 cores, NTFF profile): ~80 us end-to-end; PE dense region ~57 us
(~233 ns per N=512 matmul vs the 213 ns stream floor), bitwise-exact output.
"""

from dataclasses import dataclass, replace

import numpy as np

import concourse.mybir as mybir
from concourse import bacc
from concourse.bass_utils import run_bass_kernel_spmd
from concourse.tile import TileContext

B, M, N, K = 32, 512, 512, 2048
NCORES = 8
BPC = B // NCORES  # batches per core
KP = K // 2  # uint16 pair-columns per row
PART = 128
NCHUNK = KP // PART  # transposed chunks per operand-batch (8)
KTILES = 2 * NCHUNK  # k-tiles of 128 per batch (16)


@dataclass(frozen=True)
class Cfg:
    stage_bufs: int = 3  # per tag [128, 4096]u16 staging tiles (8 KiB/partition)
    conv_bufs: int = 20  # per tag [128, 1024]bf16 k-tiles (2 KiB/partition)
    split0: int = 2  # transpose pieces for batch 0
    split: int = 2  # transpose pieces for later batches
    alt0: bool = False  # alternate conv engines on batch 0
    store_eng: str = "gpsimd"
    conv_chunks: int = 1  # chunks converted per copy instruction
    gpsimd_convs: bool = False  # route some b conversions to GPSIMD
    obuf_bufs: int = 8
    psum_bufs: int = 8
    mm_order: str = "m_major"  # m_major: per m-group over k-tiles; t_major: per k-tile over m-groups
    b_first: bool = False  # issue b transposes before a within each piece
    deq_alt: bool = False  # alternate dequant between DVE and ACT by m-group
    fast_first_b: bool = False  # convert batch 0's first b chunk on DVE
    last_m_major: bool = False  # last batch m-major so its early epilogues hide


VARIANTS = {
    "v5": Cfg(),
    "v7": Cfg(stage_bufs=4, split0=4, alt0=True),
    "v8": Cfg(split0=4),
    "v9": Cfg(stage_bufs=4),
    "v10": Cfg(split0=1, split=1),
    "v11": Cfg(alt0=True),
    "v12": Cfg(conv_chunks=2, conv_bufs=10),
    "v13": Cfg(conv_bufs=12),
    "v14": Cfg(conv_bufs=16),
    "v15": Cfg(gpsimd_convs=True),
    "v16": Cfg(obuf_bufs=4, psum_bufs=4),
    "v17": Cfg(store_eng="scalar"),
    "v19": Cfg(mm_order="t_major"),
    "v20": Cfg(mm_order="t_major", alt0=True),
    "v21": Cfg(mm_order="t_major", split0=4),
    "v22": Cfg(mm_order="t_major", conv_bufs=24),
    "v23": Cfg(mm_order="t_major_tail"),
    "v24": Cfg(mm_order="t_major", b_first=True),
    "v25": Cfg(mm_order="t_major", split=1),
    "v26": Cfg(mm_order="t_major", stage_bufs=2),
    "v27": Cfg(mm_order="t_major", deq_alt=True, fast_first_b=True),
    "v29": Cfg(mm_order="t_major", deq_alt=True, fast_first_b=True, last_m_major=True),
    "v30": Cfg(mm_order="t_major", last_m_major=True),
}

_cfg = VARIANTS["v29"]


def set_variant(name):
    global _cfg
    _cfg = VARIANTS[name] if isinstance(name, str) else name


def _build(alpha: float, bpc: int = BPC):
    cfg = _cfg
    nc = bacc.Bacc("TRN2", target_bir_lowering=False)
    a_d = nc.dram_tensor("a", [bpc, M, KP], mybir.dt.uint16, kind="ExternalInput")
    b_d = nc.dram_tensor("b", [bpc, N, KP], mybir.dt.uint16, kind="ExternalInput")
    o_d = nc.dram_tensor("out", [bpc, M, N], mybir.dt.bfloat16, kind="ExternalOutput")

    with TileContext(nc) as tc:
        with (
            tc.tile_pool(name="stage", bufs=cfg.stage_bufs) as stage,
            tc.tile_pool(name="conv", bufs=cfg.conv_bufs) as conv,
            tc.tile_pool(name="obuf", bufs=cfg.obuf_bufs) as obuf,
            tc.tile_pool(name="psum", bufs=cfg.psum_bufs, space="PSUM") as psum_pool,
        ):
            store_ring = getattr(nc, cfg.store_eng)
            for bi in range(bpc):
                ktiles = {"a": [], "b": []}
                sts = {}
                stt = {}
                for name in ("a", "b"):
                    st = stage.tile([PART, NCHUNK * M], mybir.dt.uint16, tag=f"st_{name}")
                    stt[name] = st
                    sts[name] = st[:, :].bitcast(mybir.dt.int8)  # [128, 2*KP]
                # Split transposes into pieces, alternating a/b, so the PE's
                # first k-tiles (which need both operands) arrive sooner.
                pieces = cfg.split0 if bi == 0 else cfg.split
                cs = NCHUNK // pieces  # chunks per piece
                t_order = (("b", b_d), ("a", a_d)) if cfg.b_first else (("a", a_d), ("b", b_d))
                for h in range(pieces):
                    for name, dram in t_order:
                        nc.sync.dma_start_transpose(
                            stt[name][:, h * cs * M : (h + 1) * cs * M].rearrange(
                                "q (c m) -> q c m", m=M
                            ),
                            dram[bi, :, h * cs * PART : (h + 1) * cs * PART],
                        )
                cc = cfg.conv_chunks
                for c0 in range(0, NCHUNK, cc):
                    for name in ("a", "b"):
                        chunk8 = sts[name][:, c0 * 2 * M : (c0 + cc) * 2 * M]
                        if cfg.alt0 and bi == 0:
                            eng = nc.vector if (c0 % 2 == 0) == (name == "a") else nc.scalar
                        elif name == "a":
                            eng = nc.vector
                        elif cfg.fast_first_b and bi == 0 and c0 == 0:
                            eng = nc.vector
                        elif cfg.gpsimd_convs and c0 % 4 == 3:
                            eng = nc.gpsimd
                        else:
                            eng = nc.scalar
                        bt = conv.tile(
                            [PART, cc * 2 * M], mybir.dt.bfloat16, tag=f"bf_{name}"
                        )
                        # in: [q][c][m][p] bytes -> iterate (c, p, m); out [c][p][m]
                        in_ap = chunk8.rearrange("q (c m p) -> q c p m", p=2, m=M)
                        out_ap = bt[:, :].rearrange("q (c p m) -> q c p m", m=M, p=2)
                        if eng is nc.scalar:
                            eng.copy(out=out_ap, in_=in_ap)
                        else:
                            eng.tensor_copy(out=out_ap, in_=in_ap)
                        for j in range(cc):
                            ktiles[name].append(bt[:, j * 2 * M : (j + 1) * 2 * M])
                n_mt = M // PART

                def mm(ps, mi, c, p, t):
                    nc.tensor.matmul(
                        ps[:, :],
                        ktiles["a"][c][:, p * M + mi * PART : p * M + (mi + 1) * PART],
                        ktiles["b"][c][:, p * N : (p + 1) * N],
                        start=(t == 0),
                        stop=(t == KTILES - 1),
                    )

                def epilogue(ps, mi):
                    ot = obuf.tile([PART, N], mybir.dt.bfloat16)
                    if cfg.deq_alt and mi % 2 == 1:
                        nc.scalar.activation(
                            ot[:, :],
                            ps[:, :],
                            mybir.ActivationFunctionType.Copy,
                            scale=float(alpha),
                        )
                    else:
                        nc.vector.tensor_scalar_mul(ot[:, :], ps[:, :], float(alpha))
                    store_ring.dma_start(o_d[bi, mi * PART : (mi + 1) * PART, :], ot[:, :])

                if cfg.mm_order == "m_major" or (cfg.last_m_major and bi == bpc - 1):
                    for mi in range(n_mt):
                        ps = psum_pool.tile([PART, N], mybir.dt.float32)
                        for t in range(KTILES):
                            mm(ps, mi, t // 2, t % 2, t)
                        epilogue(ps, mi)
                else:
                    pss = [
                        psum_pool.tile([PART, N], mybir.dt.float32, name=f"ps_{bi}_{mi}", tag="ps")
                        for mi in range(n_mt)
                    ]
                    for t in range(KTILES - 1):
                        for mi in range(n_mt):
                            mm(pss[mi], mi, t // 2, t % 2, t)
                    t = KTILES - 1
                    if cfg.mm_order == "t_major_tail":
                        # finish each m-group's accumulation and immediately
                        # emit its dequant+store so the epilogues overlap the
                        # remaining groups' final matmuls
                        for mi in range(n_mt):
                            mm(pss[mi], mi, t // 2, t % 2, t)
                            epilogue(pss[mi], mi)
                    else:
                        for mi in range(n_mt):
                            mm(pss[mi], mi, t // 2, t % 2, t)
                        for mi in range(n_mt):
                            epilogue(pss[mi], mi)
    nc.compile()
    return nc


def run(a, b, alpha, trace=False, repeats=1):
    """Run on 8 NeuronCores; returns (out [32,512,512] bf16, list[BassKernelResults])."""
    a = np.ascontiguousarray(np.asarray(a))
    b = np.ascontiguousarray(np.asarray(b))
    if a.dtype != np.int8:
        a = a.astype(np.int8)
    if b.dtype != np.int8:
        b = b.astype(np.int8)
    nc = _build(float(alpha))
    in_maps = []
    for ci in range(NCORES):
        sl = slice(ci * BPC, (ci + 1) * BPC)
        in_maps.append({"a": a[sl].view(np.uint16), "b": b[sl].view(np.uint16)})
    all_res = []
    for _ in range(repeats):
        res = run_bass_kernel_spmd(
            nc, in_maps, core_ids=list(range(NCORES)), trace=trace
        )
        all_res.append(res)
    out = np.concatenate([r["out"] for r in all_res[-1].results], axis=0)
    return out, all_res


def kernel(a, b, alpha):
    out, _ = run(a, b, alpha)
    return out
